# revision 24
# baseline (speedup 1.0000x reference)
"""Trainium2 Bass kernel for nn_CrossInferenceBlock (bilinear cross attention).

Computation (T=256, S=256, F=1024, A=256):
    theta = (x @ a_w + a_b).reshape(T, S, A)
    phi   = (x @ b_w + b_b).reshape(T, S, A)
    feats = (x @ g_w + g_b).reshape(T, S, F)
    attn  = einsum("tsa,tra->tsr", theta, phi)
    out   = einsum("tsr,trf->tsf", attn, feats) / (S + T)

Sharding: data-parallel over t — each of the 8 cores takes 32 contiguous
t-slices; the Linear weights are replicated.

Layout strategy (no on-chip transposes needed):
    - x arrives pre-transposed per t-slice (F on partitions).
    - thetaT/phiT are produced A-on-partitions (lhsT = a_w/b_w natural).
    - attnT[r, s] is produced r-on-partitions (lhsT = phiT, rhs = thetaT).
    - feats is produced naturally s-on-partitions (lhsT = xT slice, rhs = g_w),
      with g_b folded in via a K=1 ones-row matmul.
    - out[s, f] comes out naturally (lhsT = attnT, rhs = feats); the 1/(S+T)
      scale is folded into the attnT PSUM->SBUF copy.

Matmuls run in fp16 (fp32 PSUM accumulation): measured end-to-end rel l2
error vs the fp32 reference is ~6e-4 (bf16 would be ~5e-3).
"""

import numpy as np

import concourse.bass as bass
import concourse.bacc as bacc
import concourse.tile as tile
from concourse import mybir
from concourse.bass_utils import run_bass_kernel_spmd

T, S, F, A = 256, 256, 1024, 256
N_CORES = 8
T_LOC = T // N_CORES          # 32 t-slices per core
P = 128
KT = F // P                   # 8 contraction tiles over F
MT_A = A // P                 # 2 output tiles over A
MT_S = S // P                 # 2 tiles over s (rows of one t-slice)
NF = 512                      # matmul free-dim chunk for F-wide outputs
NC_F = F // NF                # 2 chunks
TG = 4                        # t-slices fetched per input DMA
NG = T_LOC // TG              # 8 DMA groups per core
OUT_SCALE = 1.0 / (S + T)

F16 = mybir.dt.float16
F32 = mybir.dt.float32

_COMPILED = None


def _build():
    nc = bacc.Bacc("TRN2", target_bir_lowering=False, debug=False)

    # All inputs are host-prearranged so every DMA reads per-partition
    # CONTIGUOUS runs (4-16KB), keeping HBM transfers at full rate.
    # x: (NG, P, KT, TG, S) with t = g*TG + ti, f = kt*P + p.
    x_d = nc.dram_tensor("x", [NG, P, KT, TG, S], F16, kind="ExternalInput")
    aw_d = nc.dram_tensor("aw", [P, KT, MT_A, P], F16, kind="ExternalInput")
    bw_d = nc.dram_tensor("bw", [P, KT, MT_A, P], F16, kind="ExternalInput")
    gw_d = nc.dram_tensor("gw", [P, KT, F], F16, kind="ExternalInput")
    ab_d = nc.dram_tensor("ab", [A], F32, kind="ExternalInput")
    bb_d = nc.dram_tensor("bb", [A], F32, kind="ExternalInput")
    gb_d = nc.dram_tensor("gb", [F], F32, kind="ExternalInput")
    out_d = nc.dram_tensor("out", [T_LOC, S, F], F32, kind="ExternalOutput")

    x_ap = x_d.ap()
    aw_ap = aw_d.ap()
    bw_ap = bw_d.ap()
    gw_ap = gw_d.ap()
    ab_ap = ab_d.ap().rearrange("(mt p) -> p mt", p=P)
    bb_ap = bb_d.ap().rearrange("(mt p) -> p mt", p=P)
    out_ap = out_d.ap()

    with tile.TileContext(nc) as tc:
        with (
            tc.tile_pool(name="const", bufs=1) as const,
            tc.tile_pool(name="xin", bufs=3) as xin,
            tc.tile_pool(name="proj", bufs=6) as proj,
            tc.tile_pool(name="fsb", bufs=3) as fsb,
            tc.tile_pool(name="asb", bufs=3) as asb,
            tc.tile_pool(name="osb", bufs=6) as osb,
            tc.tile_pool(name="ps_s", bufs=4, space="PSUM") as ps_s,
            tc.tile_pool(name="ps_b", bufs=4, space="PSUM") as ps_b,
        ):
            # DMA issue order matters at startup: get the operands of the
            # first t-slice's matmuls (aw/ab, x[g=0], bw/bb) in before the
            # big g_w load so the PE starts ~3us in instead of ~15us.
            xt0 = xin.tile([P, KT, TG, S], F16, tag="xt")
            nc.sync.dma_start(out=xt0[:], in_=x_ap[0])
            aw_sb = const.tile([P, KT, MT_A, P], F16)
            nc.sync.dma_start(out=aw_sb[:], in_=aw_ap)
            ab_sb = const.tile([P, MT_A], F32)
            nc.sync.dma_start(out=ab_sb[:], in_=ab_ap)
            bw_sb = const.tile([P, KT, MT_A, P], F16)
            nc.sync.dma_start(out=bw_sb[:], in_=bw_ap)
            bb_sb = const.tile([P, MT_A], F32)
            nc.sync.dma_start(out=bb_sb[:], in_=bb_ap)
            gw_sb = const.tile([P, KT, F], F16)
            nc.sync.dma_start(out=gw_sb[:], in_=gw_ap)
            gbb_sb = const.tile([P, F], F32)
            gb_bcast = bass.AP(
                tensor=gb_d.ap().tensor,
                offset=gb_d.ap().offset,
                ap=[[0, P], [1, F]],
            )
            nc.sync.dma_start(out=gbb_sb[:], in_=gb_bcast)

            for g in range(NG):
                if g == 0:
                    xt = xt0
                else:
                    xt = xin.tile([P, KT, TG, S], F16, tag="xt")
                    nc.sync.dma_start(out=xt[:], in_=x_ap[g])

                for ti in range(TG):
                    t = g * TG + ti

                    # thetaT/phiT: [A on partitions, s free], + bias, -> fp16
                    thetaT = proj.tile([P, MT_A, S], F16, tag="thetaT")
                    phiT = proj.tile([P, MT_A, S], F16, tag="phiT")
                    for w_sb, b_sb, dst in (
                        (aw_sb, ab_sb, thetaT),
                        (bw_sb, bb_sb, phiT),
                    ):
                        for mt in range(MT_A):
                            ps = ps_s.tile([P, S], F32, tag="ps_s")
                            for kt in range(KT):
                                nc.tensor.matmul(
                                    ps[:],
                                    lhsT=w_sb[:, kt, mt, :],
                                    rhs=xt[:, kt, ti, :],
                                    start=(kt == 0),
                                    stop=(kt == KT - 1),
                                )
                            nc.vector.tensor_scalar_add(
                                dst[:, mt, :], ps[:], b_sb[:, mt : mt + 1]
                            )

                    # attnT[r, s] = sum_a phi[r, a] theta[s, a]; scale folded in
                    attnT = asb.tile([P, MT_S, S], F16, tag="attnT")
                    for rt in range(MT_S):
                        ps = ps_s.tile([P, S], F32, tag="ps_s")
                        for kt in range(MT_A):
                            nc.tensor.matmul(
                                ps[:],
                                lhsT=phiT[:, kt, rt * P : (rt + 1) * P],
                                rhs=thetaT[:, kt, :],
                                start=(kt == 0),
                                stop=(kt == MT_A - 1),
                            )
                        nc.scalar.activation(
                            out=attnT[:, rt, :],
                            in_=ps[:],
                            func=mybir.ActivationFunctionType.Copy,
                            scale=OUT_SCALE,
                        )

                    # feats: [s on partitions, f free]; g_b added on DVE
                    # during the PSUM->SBUF eviction (a K=1 bias matmul
                    # would cost a full N-column stream on the PE).
                    feats = fsb.tile([P, MT_S, F], F16, tag="feats")
                    for mt in range(MT_S):
                        for c in range(NC_F):
                            ps = ps_b.tile([P, NF], F32, tag="ps_b")
                            for kt in range(KT):
                                nc.tensor.matmul(
                                    ps[:],
                                    lhsT=xt[:, kt, ti, mt * P : (mt + 1) * P],
                                    rhs=gw_sb[:, kt, c * NF : (c + 1) * NF],
                                    start=(kt == 0),
                                    stop=(kt == KT - 1),
                                )
                            nc.vector.tensor_add(
                                feats[:, mt, c * NF : (c + 1) * NF],
                                ps[:],
                                gbb_sb[:, c * NF : (c + 1) * NF],
                            )

                    # out[s, f] = sum_r attnT[r, s] feats[r, f]; stores issue
                    # per 512-chunk so the last DMA starts one eviction earlier
                    for mt in range(MT_S):
                        out_sb = osb.tile([P, F], F32, tag="out_sb")
                        for c in range(NC_F):
                            ps = ps_b.tile([P, NF], F32, tag="ps_b")
                            for rt in range(MT_S):
                                nc.tensor.matmul(
                                    ps[:],
                                    lhsT=attnT[:, rt, mt * P : (mt + 1) * P],
                                    rhs=feats[:, rt, c * NF : (c + 1) * NF],
                                    start=(rt == 0),
                                    stop=(rt == MT_S - 1),
                                )
                            nc.vector.tensor_copy(
                                out_sb[:, c * NF : (c + 1) * NF], ps[:]
                            )
                            nc.sync.dma_start(
                                out=out_ap[
                                    t, mt * P : (mt + 1) * P, c * NF : (c + 1) * NF
                                ],
                                in_=out_sb[:, c * NF : (c + 1) * NF],
                            )

    nc.compile()
    return nc


def _get_compiled():
    global _COMPILED
    if _COMPILED is None:
        _COMPILED = _build()
    return _COMPILED


def _prep_inputs(inputs):
    x = np.asarray(inputs["batch_data"], dtype=np.float32)
    assert x.shape == (T * S, F), x.shape
    # (T, S, F) -> per-core (T_LOC, F, S) -> (NG, TG, KT, P, S) -> (NG, P, KT, TG, S)
    x16 = (
        x.reshape(T, S, F)
        .transpose(0, 2, 1)
        .astype(np.float16)
        .reshape(N_CORES, NG, TG, KT, P, S)
        .transpose(0, 1, 4, 3, 2, 5)
    )
    x16 = np.ascontiguousarray(x16)

    def tile_w(w, mt):  # (F, N) -> (P, KT, mt, 128)
        n = w.shape[1]
        return np.ascontiguousarray(
            w.astype(np.float16).reshape(KT, P, mt, n // mt).transpose(1, 0, 2, 3)
        )

    aw16 = tile_w(np.asarray(inputs["a_w"], np.float32), MT_A)
    bw16 = tile_w(np.asarray(inputs["b_w"], np.float32), MT_A)
    gw16 = tile_w(np.asarray(inputs["g_w"], np.float32), 1).reshape(P, KT, F)
    ab32 = np.ascontiguousarray(np.asarray(inputs["a_b"], np.float32))
    bb32 = np.ascontiguousarray(np.asarray(inputs["b_b"], np.float32))
    gb32 = np.ascontiguousarray(np.asarray(inputs["g_b"], np.float32))
    in_maps = []
    for c in range(N_CORES):
        in_maps.append(
            {
                "x": x16[c],
                "aw": aw16,
                "bw": bw16,
                "gw": gw16,
                "ab": ab32,
                "bb": bb32,
                "gb": gb32,
            }
        )
    return in_maps


def run_spmd(inputs, **kwargs):
    """Run the compiled kernel; returns (full_output, BassKernelResults)."""
    nc = _get_compiled()
    in_maps = _prep_inputs(inputs)
    res = run_bass_kernel_spmd(nc, in_maps, list(range(N_CORES)), **kwargs)
    out = np.concatenate(
        [np.asarray(res.results[c]["out"], np.float32) for c in range(N_CORES)],
        axis=0,
    )
    return out, res


def kernel(**inputs) -> np.ndarray:
    out, _ = run_spmd(inputs)
    return out
